# revision 12
# baseline (speedup 1.0000x reference)
"""Trainium2 Bass kernel for quantized-linear + LoRA (nn_LoRALinear).

Computes, for x:(4,2048,4096) f32, weight_quant:(4096,4096) i32 in [0,16),
scale/zero:(4096,1) f32, lora_A:(16,4096), lora_B:(4096,16), bias:(4096,):

    W = (weight_quant - zero) * scale
    y = x @ W.T + bias + 2.0 * (x @ lora_A.T) @ lora_B.T

Sharding across 8 NeuronCores: 4-way over tokens x 2-way over out-features.
Per core: x-slice (2048, 4096), weight rows slice (2048 of 4096), output
block (2048 tokens, 2048 features); host only slices inputs / stitches blocks.

Device algorithm (per core):

    P[o,n]   = sum_d (wq[o,d]-8) * x[n,d]          (PE; fp8e4 weights (exact
                                                    ints) x bf16 moving)
             + sum_r B2[o,r] * t[r,n]              (K=17 fp32r matmul into the
             + (8-zero[o]) * rowsum[n]              same psum accumulation)
    y[n,o]   = scale[o] * P[o,n] + bias[o]         (ScalarE psum eviction)

with t = lora_A @ x.T augmented by a ones-row giving rowsum, B2 = 2*lora_B/
scale. Output lands transposed [o,n]; PE de-transposes before DMA-out.
"""
import os
import sys
import types

sys.path.insert(0, "/opt/trn_rl_repo")

import numpy as np

import concourse.bass as bass
import concourse.mybir as mybir
import concourse.tile as tile
from concourse import bacc
from concourse.bass_utils import run_bass_kernel_spmd
from concourse.masks import make_identity

F32 = mybir.dt.float32
F32R = mybir.dt.float32r
BF16 = mybir.dt.bfloat16
FP8 = mybir.dt.float8e4
I32 = mybir.dt.int32

# Problem shape (hardcoded per contract)
B, S, D, O, R = 4, 2048, 4096, 4096, 16
SCALING = 32.0 / 16.0
N_TOK = B * S            # 8192 tokens
T_SH, F_SH = 4, 2        # token shards x feature shards = 8 cores
N_SH = N_TOK // T_SH     # 2048 tokens per core
O_SH = O // F_SH         # 2048 out-features per core

NT = 4                   # n tiles per core
N_TILE = N_SH // NT      # 512
KC = D // 128            # 32 contraction chunks
OT = O_SH // 128         # 16 o tiles
OQ = 4                   # o tiles per psum pass
WQ_CENTER = 8.0          # center wq (exact in fp8e4; smaller dot magnitude)


def _ensure_ntff_hook():
    """Best-effort: register the axon NTFF profile hook so trace=True works."""
    try:
        import antenv
        if "antenv.axon_hooks" not in sys.modules:
            hooks_mod = types.ModuleType("antenv.axon_hooks")
            hooks_mod._hook = None
            hooks_mod.set_axon_ntff_profile_hook = lambda h: setattr(hooks_mod, "_hook", h)
            hooks_mod.get_axon_ntff_profile_hook = lambda: hooks_mod._hook
            sys.modules["antenv.axon_hooks"] = hooks_mod
            antenv.axon_hooks = hooks_mod
        from trn_agent_boot.trn_boot import _ntff_profile_via_ctypes
        sys.modules["antenv.axon_hooks"].set_axon_ntff_profile_hook(
            _ntff_profile_via_ctypes("/opt/axon/libaxon_pjrt.so")
        )
        import concourse.bass_utils as bu
        bu.upload_artifacts = lambda tmpdir: tmpdir
    except Exception:
        pass


def build_nc() -> bass.Bass:
    nc = bacc.Bacc("TRN2", target_bir_lowering=False, debug=False)

    x_d = nc.dram_tensor("x", (N_SH, D), F32, kind="ExternalInput")
    wq_d = nc.dram_tensor("wq", (O_SH, D), I32, kind="ExternalInput")
    scale_d = nc.dram_tensor("scale", (O_SH,), F32, kind="ExternalInput")
    zero_d = nc.dram_tensor("zero", (O_SH,), F32, kind="ExternalInput")
    bias_d = nc.dram_tensor("bias", (O_SH,), F32, kind="ExternalInput")
    a_d = nc.dram_tensor("lora_a", (R, D), F32, kind="ExternalInput")
    b_d = nc.dram_tensor("lora_b", (O_SH, R), F32, kind="ExternalInput")
    y_d = nc.dram_tensor("y", (N_SH, O_SH), F32, kind="ExternalOutput")

    with tile.TileContext(nc) as tc:
        with (
            tc.tile_pool(name="const", bufs=1) as cpool,
            tc.tile_pool(name="wt", bufs=1) as wtpool,
            tc.tile_pool(name="xt", bufs=2) as xtpool,
            tc.tile_pool(name="stage", bufs=3) as stage,
            tc.tile_pool(name="cvt", bufs=2) as cvt,
            tc.tile_pool(name="outp", bufs=3) as outp,
            tc.tile_pool(name="ps_small", bufs=3, space="PSUM") as ps_small,
            tc.tile_pool(name="ps_t", bufs=1, space="PSUM") as ps_tp,
            tc.tile_pool(name="ps_acc", bufs=4, space="PSUM") as ps_accp,
        ):
            # ---------------- constants ----------------
            ident_b = cpool.tile([128, 128], BF16)
            make_identity(nc, ident_b)
            ident_f = cpool.tile([128, 128], F32)
            make_identity(nc, ident_f)
            ident_r = cpool.tile([128, 128], F32R)
            nc.vector.tensor_copy(ident_r[:], ident_f[:])

            # scale/bias/zero as [128 partitions, 16 o-tiles] f32
            scale_sb = cpool.tile([128, OT], F32)
            bias_sb = cpool.tile([128, OT], F32)
            zero_sb = cpool.tile([128, OT], F32)
            nc.sync.dma_start(scale_sb[:], scale_d.rearrange("(t p) -> p t", p=128))
            nc.sync.dma_start(bias_sb[:], bias_d.rearrange("(t p) -> p t", p=128))
            nc.sync.dma_start(zero_sb[:], zero_d.rearrange("(t p) -> p t", p=128))
            rcp_sb = cpool.tile([128, OT], F32)
            nc.vector.reciprocal(rcp_sb[:], scale_sb[:])
            rcp2_sb = cpool.tile([128, OT], F32)
            nc.vector.tensor_scalar_mul(rcp2_sb[:], rcp_sb[:], float(SCALING))

            # B2augT [18, OT, 128] fp32r: rows 0..15 = (2*B/scale).T,
            # row 16 = (WQ_CENTER - zero)  [pairs with rowsum row of t_aug],
            # row 17 = bias/scale          [pairs with the ones row of t_sb]
            b2augT = cpool.tile([18, OT, 128], F32R)
            for t in range(OT):
                bblk = stage.tile([128, R], F32, tag="bblk")
                nc.sync.dma_start(bblk[:], b_d[t * 128:(t + 1) * 128, :])
                pre = stage.tile([128, 18], F32R, tag="pre")
                nc.vector.tensor_scalar(
                    out=pre[:, 0:R], in0=bblk[:],
                    scalar1=rcp2_sb[:, t:t + 1], scalar2=None,
                    op0=mybir.AluOpType.mult,
                )
                nc.vector.tensor_scalar(
                    out=pre[:, R:R + 1], in0=zero_sb[:, t:t + 1],
                    scalar1=-1.0, scalar2=float(WQ_CENTER),
                    op0=mybir.AluOpType.mult, op1=mybir.AluOpType.add,
                )
                nc.vector.tensor_mul(
                    pre[:, R + 1:R + 2], bias_sb[:, t:t + 1], rcp_sb[:, t:t + 1]
                )
                psb = ps_small.tile([18, 128], F32R, tag="ps_sm")
                nc.tensor.transpose(psb[:], pre[:], ident_r[:])
                nc.vector.tensor_copy(b2augT[:, t, :], psb[:].bitcast(F32))

            # A_augT [128, KC, 17] bf16: cols 0..15 = A.T chunk, col16 = ones
            a_augT = cpool.tile([128, KC, R + 1], BF16)
            nc.gpsimd.memset(a_augT[:, :, R:R + 1], 1.0)
            ones32 = cpool.tile([32, N_TILE], F32)
            nc.gpsimd.memset(ones32[:], 1.0)
            for k in range(KC):
                a_st = stage.tile([R, 128], F32, tag="a_st")
                nc.sync.dma_start(a_st[:], a_d[:, k * 128:(k + 1) * 128])
                a_bf = cvt.tile([R, 128], BF16, tag="a_bf")
                nc.vector.tensor_copy(a_bf[:], a_st[:])
                psa = ps_small.tile([128, R], BF16, tag="ps_sm")
                nc.tensor.transpose(psa[:], a_bf[:], ident_b[0:R, 0:R])
                nc.vector.tensor_copy(a_augT[:, k, 0:R], psa[:])

            # x rows are cast-DMA'd (f32 -> bf16) straight into SBUF tiles and
            # transposed on the PE into xT.
            x_cast_tiles = {}

            def emit_x_cast(nt):
                tiles = []
                for g in range(N_TILE // 128):
                    r0 = nt * N_TILE + g * 128
                    xc = cvt.tile([128, D], BF16, tag="xcast")
                    nc.gpsimd.dma_start(xc[:], x_d[r0:r0 + 128, :])
                    tiles.append(xc)
                x_cast_tiles[nt] = tiles

            # ------- Wt: transposed centered weights, fp8e4 (exact), resident -------
            # wt_og[og][p=d_in, k, oi, o_in] = wq[(og*4+oi)*128+o_in, k*128+p] - 8
            # Split into OQ separate tiles; builds are emitted interleaved with
            # the first n-tile's compute so the PE never queues idle behind them.
            wt_og = []
            for og in range(OQ):
                wt_g_tile = wtpool.tile([128, KC, OQ, 128], FP8, tag=f"wt{og}")
                wt_og.append(wt_g_tile)

            def emit_og_build(og):
                wt_g = wt_og[og]
                for rg in range(4):
                    wqc = cvt.tile([128, D], BF16, tag="wqcast")
                    nc.gpsimd.dma_start(
                        wqc[:], wq_d[og * 512 + rg * 128: og * 512 + (rg + 1) * 128, :]
                    )
                    for k in range(KC):
                        pst = ps_small.tile([128, 128], BF16, tag="ps_sm")
                        nc.tensor.transpose(
                            pst[:], wqc[:, k * 128:(k + 1) * 128], ident_b[:]
                        )
                        # center by -8 during the psum eviction (bf16 -> fp8)
                        nc.vector.tensor_scalar(
                            out=wt_g[:, k, rg, :], in0=pst[:],
                            scalar1=-WQ_CENTER, scalar2=None,
                            op0=mybir.AluOpType.add,
                        )

            # ---------------- main loop ----------------
            def emit_nt_prep(nt):
                # xT bf16 [128, KC, N_TILE] via PE transposes of the cast rows
                xT = xtpool.tile([128, KC, N_TILE], BF16, tag="xT")
                for g, xc in enumerate(x_cast_tiles.pop(nt)):
                    for k in range(KC):
                        pst = ps_small.tile([128, 128], BF16, tag="ps_sm")
                        nc.tensor.transpose(
                            pst[:], xc[:, k * 128:(k + 1) * 128], ident_b[:]
                        )
                        nc.vector.tensor_copy(
                            xT[:, k, g * 128:(g + 1) * 128], pst[:]
                        )
                # t_aug [17, N_TILE] psum: rows 0..15 = A@x.T, row16 = rowsum
                ps_t = ps_tp.tile([R + 1, N_TILE], F32)
                for k in range(KC):
                    nc.tensor.matmul(
                        ps_t[:], a_augT[:, k, :], xT[:, k, :],
                        start=(k == 0), stop=(k == KC - 1),
                    )
                # t_sb rows 0..16 = t_aug, row 17 = 1.0 (ones base, overwrite)
                t_sb = outp.tile([32, N_TILE], F32R, tag="t_sb")
                nc.vector.tensor_copy(t_sb[:], ones32[:])
                nc.vector.tensor_copy(t_sb[0:R + 1, :], ps_t[:])
                return xT, t_sb

            def emit_nt_oq(nt, oq, xT, t_sb):
                accs = []
                for _oi in range(OQ):
                    acc_tile = ps_accp.tile([128, N_TILE], F32, tag="acc")
                    accs.append(acc_tile)
                for k in range(KC):
                    for oi in range(OQ):
                        nc.tensor.matmul(
                            accs[oi][:], wt_og[oq][:, k, oi, :], xT[:, k, :],
                            start=(k == 0), stop=False,
                        )
                for oi in range(OQ):
                    ot = oq * OQ + oi
                    # lora + zero-correction + bias: K=18 fp32r matmul
                    nc.tensor.matmul(
                        accs[oi][:], b2augT[:, ot, :], t_sb[0:18, :],
                        start=False, stop=True,
                    )
                    # yT tile = scale[o]*P  (bias folded into the K=18 matmul)
                    yT_sb = outp.tile([128, N_TILE], F32, tag="yT")
                    nc.scalar.activation(
                        yT_sb[:], accs[oi][:],
                        mybir.ActivationFunctionType.Copy,
                        scale=scale_sb[:, ot:ot + 1],
                    )
                    # de-transpose [o,n] -> [n,o]; store
                    yst = outp.tile([128, N_TILE // 128, 128], F32, tag="yst")
                    for sub in range(N_TILE // 128):
                        psd = ps_small.tile([128, 128], F32, tag="ps_sm")
                        nc.tensor.transpose(
                            psd[:], yT_sb[:, sub * 128:(sub + 1) * 128],
                            ident_f[:],
                        )
                        nc.vector.tensor_copy(yst[:, sub, :], psd[:])
                    nc.sync.dma_start(
                        y_d[nt * N_TILE:(nt + 1) * N_TILE,
                            ot * 128:(ot + 1) * 128]
                        .rearrange("(s p) f -> p s f", p=128),
                        yst[:],
                    )

            # interleaved emission: og builds and x casts slotted between the
            # first n-tile's compute phases so the PE queue never stalls on DMA
            emit_og_build(0)
            emit_x_cast(0)
            xT0, t_sb0 = emit_nt_prep(0)
            emit_nt_oq(0, 0, xT0, t_sb0)
            emit_og_build(1)
            emit_nt_oq(0, 1, xT0, t_sb0)
            emit_og_build(2)
            emit_x_cast(1)
            emit_nt_oq(0, 2, xT0, t_sb0)
            emit_og_build(3)
            emit_nt_oq(0, 3, xT0, t_sb0)
            for nt in range(1, NT):
                xT, t_sb = emit_nt_prep(nt)
                emit_nt_oq(nt, 0, xT, t_sb)
                if nt + 1 < NT:
                    emit_x_cast(nt + 1)
                for oq in range(1, OQ):
                    emit_nt_oq(nt, oq, xT, t_sb)

    nc.finalize()
    return nc


_NC_CACHE: dict = {}


def _get_nc() -> bass.Bass:
    if "nc" not in _NC_CACHE:
        _ensure_ntff_hook()
        _NC_CACHE["nc"] = build_nc()
    return _NC_CACHE["nc"]


def kernel(x, weight_quant, scale, zero, lora_A, lora_B, bias):
    x = np.ascontiguousarray(np.asarray(x, dtype=np.float32)).reshape(N_TOK, D)
    weight_quant = np.asarray(weight_quant, dtype=np.int32)
    scale_f = np.asarray(scale, dtype=np.float32).reshape(O)
    zero_f = np.asarray(zero, dtype=np.float32).reshape(O)
    bias_f = np.asarray(bias, dtype=np.float32).reshape(O)
    lora_A = np.ascontiguousarray(np.asarray(lora_A, dtype=np.float32))
    lora_B = np.ascontiguousarray(np.asarray(lora_B, dtype=np.float32))

    nc = _get_nc()

    in_maps = []
    for core in range(T_SH * F_SH):
        ti, fi = core % T_SH, core // T_SH
        osl = slice(fi * O_SH, (fi + 1) * O_SH)
        in_maps.append({
            "x": np.ascontiguousarray(x[ti * N_SH:(ti + 1) * N_SH]),
            "wq": np.ascontiguousarray(weight_quant[osl]),
            "scale": np.ascontiguousarray(scale_f[osl]),
            "zero": np.ascontiguousarray(zero_f[osl]),
            "bias": np.ascontiguousarray(bias_f[osl]),
            "lora_a": lora_A,
            "lora_b": np.ascontiguousarray(lora_B[osl]),
        })

    trace = bool(os.environ.get("BASS_KERNEL_TRACE"))
    res = run_bass_kernel_spmd(
        nc, in_maps, core_ids=list(range(T_SH * F_SH)), trace=trace,
    )
    if trace:
        _NC_CACHE["last_exec_time_ns"] = res.exec_time_ns
        _NC_CACHE["last_results"] = res

    y = np.empty((N_TOK, O), dtype=np.float32)
    for core in range(T_SH * F_SH):
        ti, fi = core % T_SH, core // T_SH
        y[ti * N_SH:(ti + 1) * N_SH, fi * O_SH:(fi + 1) * O_SH] = \
            res.results[core]["y"]
    return y.reshape(B, S, O)


# revision 13
# speedup vs baseline: 1.0316x; 1.0316x over previous
"""Trainium2 Bass kernel for quantized-linear + LoRA (nn_LoRALinear).

Computes, for x:(4,2048,4096) f32, weight_quant:(4096,4096) i32 in [0,16),
scale/zero:(4096,1) f32, lora_A:(16,4096), lora_B:(4096,16), bias:(4096,):

    W = (weight_quant - zero) * scale
    y = x @ W.T + bias + 2.0 * (x @ lora_A.T) @ lora_B.T

Sharding across 8 NeuronCores: 4-way over tokens x 2-way over out-features.
Per core: x-slice (2048, 4096), weight rows slice (2048 of 4096), output
block (2048 tokens, 2048 features); host only slices inputs / stitches blocks.

Device algorithm (per core):

    P[o,n]   = sum_d (wq[o,d]-8) * x[n,d]          (PE; fp8e4 weights (exact
                                                    ints) x bf16 moving)
             + sum_r B2[o,r] * t[r,n]              (K=17 fp32r matmul into the
             + (8-zero[o]) * rowsum[n]              same psum accumulation)
    y[n,o]   = scale[o] * P[o,n] + bias[o]         (ScalarE psum eviction)

with t = lora_A @ x.T augmented by a ones-row giving rowsum, B2 = 2*lora_B/
scale. Output lands transposed [o,n]; PE de-transposes before DMA-out.
"""
import os
import sys
import types

sys.path.insert(0, "/opt/trn_rl_repo")

import numpy as np

import concourse.bass as bass
import concourse.mybir as mybir
import concourse.tile as tile
from concourse import bacc
from concourse.bass_utils import run_bass_kernel_spmd
from concourse.masks import make_identity

F32 = mybir.dt.float32
F32R = mybir.dt.float32r
BF16 = mybir.dt.bfloat16
FP8 = mybir.dt.float8e4
I32 = mybir.dt.int32

# Problem shape (hardcoded per contract)
B, S, D, O, R = 4, 2048, 4096, 4096, 16
SCALING = 32.0 / 16.0
N_TOK = B * S            # 8192 tokens
T_SH, F_SH = 4, 2        # token shards x feature shards = 8 cores
N_SH = N_TOK // T_SH     # 2048 tokens per core
O_SH = O // F_SH         # 2048 out-features per core

NT = 4                   # n tiles per core
N_TILE = N_SH // NT      # 512
KC = D // 128            # 32 contraction chunks
OT = O_SH // 128         # 16 o tiles
OQ = 4                   # o tiles per psum pass
WQ_CENTER = 8.0          # center wq (exact in fp8e4; smaller dot magnitude)


def _ensure_ntff_hook():
    """Best-effort: register the axon NTFF profile hook so trace=True works."""
    try:
        import antenv
        if "antenv.axon_hooks" not in sys.modules:
            hooks_mod = types.ModuleType("antenv.axon_hooks")
            hooks_mod._hook = None
            hooks_mod.set_axon_ntff_profile_hook = lambda h: setattr(hooks_mod, "_hook", h)
            hooks_mod.get_axon_ntff_profile_hook = lambda: hooks_mod._hook
            sys.modules["antenv.axon_hooks"] = hooks_mod
            antenv.axon_hooks = hooks_mod
        from trn_agent_boot.trn_boot import _ntff_profile_via_ctypes
        sys.modules["antenv.axon_hooks"].set_axon_ntff_profile_hook(
            _ntff_profile_via_ctypes("/opt/axon/libaxon_pjrt.so")
        )
        import concourse.bass_utils as bu
        bu.upload_artifacts = lambda tmpdir: tmpdir
    except Exception:
        pass


def build_nc() -> bass.Bass:
    nc = bacc.Bacc("TRN2", target_bir_lowering=False, debug=False)

    x_d = nc.dram_tensor("x", (N_SH, D), F32, kind="ExternalInput")
    wq_d = nc.dram_tensor("wq", (O_SH, D), I32, kind="ExternalInput")
    scale_d = nc.dram_tensor("scale", (O_SH,), F32, kind="ExternalInput")
    zero_d = nc.dram_tensor("zero", (O_SH,), F32, kind="ExternalInput")
    bias_d = nc.dram_tensor("bias", (O_SH,), F32, kind="ExternalInput")
    a_d = nc.dram_tensor("lora_a", (R, D), F32, kind="ExternalInput")
    b_d = nc.dram_tensor("lora_b", (O_SH, R), F32, kind="ExternalInput")
    y_d = nc.dram_tensor("y", (N_SH, O_SH), F32, kind="ExternalOutput")

    with tile.TileContext(nc) as tc:
        with (
            tc.tile_pool(name="const", bufs=1) as cpool,
            tc.tile_pool(name="wt", bufs=1) as wtpool,
            tc.tile_pool(name="xt", bufs=2) as xtpool,
            tc.tile_pool(name="stage", bufs=3) as stage,
            tc.tile_pool(name="cvt", bufs=2) as cvt,
            tc.tile_pool(name="outp", bufs=3) as outp,
            tc.tile_pool(name="dram", bufs=1, space="DRAM") as dpool,
            tc.tile_pool(name="ps_small", bufs=3, space="PSUM") as ps_small,
            tc.tile_pool(name="ps_t", bufs=1, space="PSUM") as ps_tp,
            tc.tile_pool(name="ps_acc", bufs=4, space="PSUM") as ps_accp,
        ):
            # ---------------- constants ----------------
            ident_b = cpool.tile([128, 128], BF16)
            make_identity(nc, ident_b)
            ident_f = cpool.tile([128, 128], F32)
            make_identity(nc, ident_f)
            ident_r = cpool.tile([128, 128], F32R)
            nc.vector.tensor_copy(ident_r[:], ident_f[:])

            # scale/bias/zero as [128 partitions, 16 o-tiles] f32
            scale_sb = cpool.tile([128, OT], F32)
            bias_sb = cpool.tile([128, OT], F32)
            zero_sb = cpool.tile([128, OT], F32)
            nc.sync.dma_start(scale_sb[:], scale_d.rearrange("(t p) -> p t", p=128))
            nc.sync.dma_start(bias_sb[:], bias_d.rearrange("(t p) -> p t", p=128))
            nc.sync.dma_start(zero_sb[:], zero_d.rearrange("(t p) -> p t", p=128))
            rcp_sb = cpool.tile([128, OT], F32)
            nc.vector.reciprocal(rcp_sb[:], scale_sb[:])
            rcp2_sb = cpool.tile([128, OT], F32)
            nc.vector.tensor_scalar_mul(rcp2_sb[:], rcp_sb[:], float(SCALING))

            # B2augT [18, OT, 128] fp32r: rows 0..15 = (2*B/scale).T,
            # row 16 = (WQ_CENTER - zero)  [pairs with rowsum row of t_aug],
            # row 17 = bias/scale          [pairs with the ones row of t_sb]
            b2augT = cpool.tile([18, OT, 128], F32R)
            for t in range(OT):
                bblk = stage.tile([128, R], F32, tag="bblk")
                nc.sync.dma_start(bblk[:], b_d[t * 128:(t + 1) * 128, :])
                pre = stage.tile([128, 18], F32R, tag="pre")
                nc.vector.tensor_scalar(
                    out=pre[:, 0:R], in0=bblk[:],
                    scalar1=rcp2_sb[:, t:t + 1], scalar2=None,
                    op0=mybir.AluOpType.mult,
                )
                nc.vector.tensor_scalar(
                    out=pre[:, R:R + 1], in0=zero_sb[:, t:t + 1],
                    scalar1=-1.0, scalar2=float(WQ_CENTER),
                    op0=mybir.AluOpType.mult, op1=mybir.AluOpType.add,
                )
                nc.vector.tensor_mul(
                    pre[:, R + 1:R + 2], bias_sb[:, t:t + 1], rcp_sb[:, t:t + 1]
                )
                psb = ps_small.tile([18, 128], F32R, tag="ps_sm")
                nc.tensor.transpose(psb[:], pre[:], ident_r[:])
                nc.vector.tensor_copy(b2augT[:, t, :], psb[:].bitcast(F32))

            # A_augT [128, KC, 17] bf16: cols 0..15 = A.T chunk, col16 = ones
            a_augT = cpool.tile([128, KC, R + 1], BF16)
            nc.gpsimd.memset(a_augT[:, :, R:R + 1], 1.0)
            ones32 = cpool.tile([32, N_TILE], F32)
            nc.gpsimd.memset(ones32[:], 1.0)
            for k in range(KC):
                a_st = stage.tile([R, 128], F32, tag="a_st")
                nc.sync.dma_start(a_st[:], a_d[:, k * 128:(k + 1) * 128])
                a_bf = cvt.tile([R, 128], BF16, tag="a_bf")
                nc.vector.tensor_copy(a_bf[:], a_st[:])
                psa = ps_small.tile([128, R], BF16, tag="ps_sm")
                nc.tensor.transpose(psa[:], a_bf[:], ident_b[0:R, 0:R])
                nc.vector.tensor_copy(a_augT[:, k, 0:R], psa[:])

            # x and wq are cast-DMA'd to bf16 DRAM scratch, then transposed by
            # the DMA xbar into SBUF -- no PE/DVE involvement on input layout.
            x_bf_s = dpool.tile([N_SH, D], BF16)
            wq_bf_s = dpool.tile([O_SH, D], BF16)

            def emit_x_cast(nt):
                for g in range(N_TILE // 128):
                    r0 = nt * N_TILE + g * 128
                    xc = cvt.tile([128, D], BF16, tag="xcast")
                    nc.gpsimd.dma_start(xc[:], x_d[r0:r0 + 128, :])
                    nc.sync.dma_start(x_bf_s[r0:r0 + 128, :], xc[:])

            def emit_wq_cast(og):
                for rg in range(4):
                    r0 = og * 512 + rg * 128
                    wqc = cvt.tile([128, D], BF16, tag="wqcast")
                    nc.gpsimd.dma_start(wqc[:], wq_d[r0:r0 + 128, :])
                    nc.sync.dma_start(wq_bf_s[r0:r0 + 128, :], wqc[:])

            # ------- Wt: transposed centered weights, fp8e4 (exact), resident -------
            # wt_og[og][p=d_in, k, oi, o_in] = wq[(og*4+oi)*128+o_in, k*128+p] - 8
            # Split into OQ separate tiles; builds are emitted interleaved with
            # the first n-tile's compute so the PE never queues idle behind them.
            wt_og = []
            for og in range(OQ):
                wt_g_tile = wtpool.tile([128, KC, OQ, 128], FP8, tag=f"wt{og}")
                wt_og.append(wt_g_tile)

            def emit_og_build(og):
                wt_g = wt_og[og]
                wqT = xtpool.tile([128, KC, 512], BF16, tag="xT")
                nc.sync.dma_start_transpose(
                    wqT[:], wq_bf_s[og * 512:(og + 1) * 512, :]
                )
                # center by -8 and narrow to fp8 in one DVE pass per k-chunk
                for k in range(KC):
                    nc.vector.tensor_scalar(
                        out=wt_g[:, k].rearrange("p a b -> p (a b)"),
                        in0=wqT[:, k],
                        scalar1=-WQ_CENTER, scalar2=None,
                        op0=mybir.AluOpType.add,
                    )

            # ---------------- main loop ----------------
            def emit_nt_prep(nt):
                # xT bf16 [128, KC, N_TILE] via one xbar DMA-transpose
                xT = xtpool.tile([128, KC, N_TILE], BF16, tag="xT")
                nc.sync.dma_start_transpose(
                    xT[:], x_bf_s[nt * N_TILE:(nt + 1) * N_TILE, :]
                )
                # t_aug [17, N_TILE] psum: rows 0..15 = A@x.T, row16 = rowsum
                ps_t = ps_tp.tile([R + 1, N_TILE], F32)
                for k in range(KC):
                    nc.tensor.matmul(
                        ps_t[:], a_augT[:, k, :], xT[:, k, :],
                        start=(k == 0), stop=(k == KC - 1),
                    )
                # t_sb rows 0..16 = t_aug, row 17 = 1.0 (ones base, overwrite)
                t_sb = outp.tile([32, N_TILE], F32R, tag="t_sb")
                nc.vector.tensor_copy(t_sb[:], ones32[:])
                nc.vector.tensor_copy(t_sb[0:R + 1, :], ps_t[:])
                return xT, t_sb

            def emit_nt_oq(nt, oq, xT, t_sb):
                accs = []
                for _oi in range(OQ):
                    acc_tile = ps_accp.tile([128, N_TILE], F32, tag="acc")
                    accs.append(acc_tile)
                for k in range(KC):
                    for oi in range(OQ):
                        nc.tensor.matmul(
                            accs[oi][:], wt_og[oq][:, k, oi, :], xT[:, k, :],
                            start=(k == 0), stop=False,
                        )
                for oi in range(OQ):
                    ot = oq * OQ + oi
                    # lora + zero-correction + bias: K=18 fp32r matmul
                    nc.tensor.matmul(
                        accs[oi][:], b2augT[:, ot, :], t_sb[0:18, :],
                        start=False, stop=True,
                    )
                    # yT tile = scale[o]*P  (bias folded into the K=18 matmul)
                    yT_sb = outp.tile([128, N_TILE], F32, tag="yT")
                    nc.scalar.activation(
                        yT_sb[:], accs[oi][:],
                        mybir.ActivationFunctionType.Copy,
                        scale=scale_sb[:, ot:ot + 1],
                    )
                    # de-transpose [o,n] -> [n,o]; store
                    yst = outp.tile([128, N_TILE // 128, 128], F32, tag="yst")
                    for sub in range(N_TILE // 128):
                        psd = ps_small.tile([128, 128], F32, tag="ps_sm")
                        nc.tensor.transpose(
                            psd[:], yT_sb[:, sub * 128:(sub + 1) * 128],
                            ident_f[:],
                        )
                        nc.vector.tensor_copy(yst[:, sub, :], psd[:])
                    nc.sync.dma_start(
                        y_d[nt * N_TILE:(nt + 1) * N_TILE,
                            ot * 128:(ot + 1) * 128]
                        .rearrange("(s p) f -> p s f", p=128),
                        yst[:],
                    )

            # interleaved emission: casts/builds slotted between the first
            # n-tile's compute phases so neither PE nor DMA queues stall
            emit_wq_cast(0)
            emit_x_cast(0)
            emit_og_build(0)
            xT0, t_sb0 = emit_nt_prep(0)
            emit_nt_oq(0, 0, xT0, t_sb0)
            emit_wq_cast(1)
            emit_og_build(1)
            emit_nt_oq(0, 1, xT0, t_sb0)
            emit_wq_cast(2)
            emit_og_build(2)
            emit_nt_oq(0, 2, xT0, t_sb0)
            emit_wq_cast(3)
            emit_og_build(3)
            emit_x_cast(1)
            emit_nt_oq(0, 3, xT0, t_sb0)
            for nt in range(1, NT):
                xT, t_sb = emit_nt_prep(nt)
                emit_nt_oq(nt, 0, xT, t_sb)
                if nt + 1 < NT:
                    emit_x_cast(nt + 1)
                for oq in range(1, OQ):
                    emit_nt_oq(nt, oq, xT, t_sb)

    nc.finalize()
    return nc


_NC_CACHE: dict = {}


def _get_nc() -> bass.Bass:
    if "nc" not in _NC_CACHE:
        _ensure_ntff_hook()
        _NC_CACHE["nc"] = build_nc()
    return _NC_CACHE["nc"]


def kernel(x, weight_quant, scale, zero, lora_A, lora_B, bias):
    x = np.ascontiguousarray(np.asarray(x, dtype=np.float32)).reshape(N_TOK, D)
    weight_quant = np.asarray(weight_quant, dtype=np.int32)
    scale_f = np.asarray(scale, dtype=np.float32).reshape(O)
    zero_f = np.asarray(zero, dtype=np.float32).reshape(O)
    bias_f = np.asarray(bias, dtype=np.float32).reshape(O)
    lora_A = np.ascontiguousarray(np.asarray(lora_A, dtype=np.float32))
    lora_B = np.ascontiguousarray(np.asarray(lora_B, dtype=np.float32))

    nc = _get_nc()

    in_maps = []
    for core in range(T_SH * F_SH):
        ti, fi = core % T_SH, core // T_SH
        osl = slice(fi * O_SH, (fi + 1) * O_SH)
        in_maps.append({
            "x": np.ascontiguousarray(x[ti * N_SH:(ti + 1) * N_SH]),
            "wq": np.ascontiguousarray(weight_quant[osl]),
            "scale": np.ascontiguousarray(scale_f[osl]),
            "zero": np.ascontiguousarray(zero_f[osl]),
            "bias": np.ascontiguousarray(bias_f[osl]),
            "lora_a": lora_A,
            "lora_b": np.ascontiguousarray(lora_B[osl]),
        })

    trace = bool(os.environ.get("BASS_KERNEL_TRACE"))
    res = run_bass_kernel_spmd(
        nc, in_maps, core_ids=list(range(T_SH * F_SH)), trace=trace,
    )
    if trace:
        _NC_CACHE["last_exec_time_ns"] = res.exec_time_ns
        _NC_CACHE["last_results"] = res

    y = np.empty((N_TOK, O), dtype=np.float32)
    for core in range(T_SH * F_SH):
        ti, fi = core % T_SH, core // T_SH
        y[ti * N_SH:(ti + 1) * N_SH, fi * O_SH:(fi + 1) * O_SH] = \
            res.results[core]["y"]
    return y.reshape(B, S, O)
